# revision 5
# baseline (speedup 1.0000x reference)
"""Trainium2 Bass kernel v2 for nn_CausalAttention (B=8, S=2048, D=1024, fp32).

Reference semantics (softmax over the QUERY axis, axis=1):
    q = x @ Wq; k = x @ Wk; v = x @ Wv          per batch  [S, D]
    scores[q_, k_] = q[q_] . k[k_], masked to -inf where k_ > q_
    w = softmax(scores, axis=q_)                 (normalize over queries per key)
    out[q_] = sum_k w[q_, k_] v[k_]

v2 (data-parallel over batch, 8 cores, no collectives). Per core:
    S^T = K Q^T = (x M') x^T with M' = Wk Wq^T   (one fewer projection GEMM)
  Prologue: Wq/Wk -> PE-transpose -> M' (fp32r, SBUF); x -> PE-transpose xT.
  Fused main loop over k-chunks kc (128 k rows):
    - every 4th kc: A'T[j, k-512-group] = M'^T @ xT        (SBUF, 2MB)
    - St[k, q] = A'T_kc^T @ xT  (q-512 groups, causal-skipped, diagonal
      group N-trimmed), diag mask, global row max, E = exp(St-M) -> bf16
      -> one whole-row DMA to DRAM; row-sums via activation accum_out;
      r = 1/sum
    - V[kc] = xT_kc^T @ Wv (psum), V''[kc] = r*V -> bf16 (SBUF resident)
    - lagged by 2: C-group j: out[q-chunk] = sum_kc E^T @ V'' (bf16)
  SBUF: one rotating 32KB-slab tag {Wq,Wk,WqT,WkT -> M',Wv,V''} keeps
  peak under the ~208KB/partition budget.
"""

import numpy as np

B, S, D = 8, 2048, 1024
P = 128
NCORES = 8
NSC = S // P   # 16 k/q chunks of 128
NDC = D // P   # 8 d-chunks
QG = 512       # B-phase q-group width
NQG = S // QG  # 4
CG = 256       # C-phase q-block width per group (2 q-chunks)
MASK_NEG = -1.0e30


def build_body(tc, out_ap, x_ap, wq_ap, wk_ap, wv_ap):
    from contextlib import ExitStack
    import concourse.mybir as mybir
    from concourse.masks import make_identity

    f32 = mybir.dt.float32
    f32r = mybir.dt.float32r
    bf16 = mybir.dt.bfloat16
    AF = mybir.ActivationFunctionType
    ALU = mybir.AluOpType
    AX = mybir.AxisListType

    nc = tc.nc

    with ExitStack() as ctx:
        dram = ctx.enter_context(tc.tile_pool(name="dram", bufs=1, space="DRAM"))
        persist = ctx.enter_context(tc.tile_pool(name="persist", bufs=1))
        w32 = ctx.enter_context(tc.tile_pool(name="w32", bufs=3))
        xck = ctx.enter_context(tc.tile_pool(name="xck", bufs=3))
        xtp = ctx.enter_context(tc.tile_pool(name="xtp", bufs=1))
        atp = ctx.enter_context(tc.tile_pool(name="atp", bufs=1))
        etp = ctx.enter_context(tc.tile_pool(name="etp", bufs=1))
        ecp = ctx.enter_context(tc.tile_pool(name="ecp", bufs=1))
        osp = ctx.enter_context(tc.tile_pool(name="osp", bufs=1))
        tiny = ctx.enter_context(tc.tile_pool(name="tiny", bufs=4))
        ps512 = ctx.enter_context(tc.tile_pool(name="ps512", bufs=8, space="PSUM"))

        e_dram = dram.tile([P, NSC, S], bf16, tag="e_d")  # E[k%128, k//128, q]

        # constants
        ident_f32 = persist.tile([P, P], f32, tag="ident_f32")
        make_identity(nc, ident_f32[:])
        ident = persist.tile([P, P], f32r, tag="ident")
        nc.vector.tensor_copy(ident[:], ident_f32[:])
        dmask = persist.tile([P, P], f32, tag="dmask")
        # dmask[k, q] = 0 where q >= k else MASK_NEG
        nc.gpsimd.memset(dmask[:], 0.0)
        nc.gpsimd.affine_select(
            out=dmask[:], in_=dmask[:], compare_op=ALU.is_ge, fill=MASK_NEG,
            base=0, pattern=[[1, P]], channel_multiplier=-1,
        )
        rall = persist.tile([P, NSC], f32, tag="rall")

        def copy_engine(i):
            return nc.scalar.copy if i % 2 == 0 else nc.vector.tensor_copy

        # PE warmup: release the HAM clock gate while the first W DMA chunk
        # is still in flight; junk transposes, never read.
        for w in range(3):
            pwu = ps512.tile([P, QG], f32r, tag="mm", name="pwu")
            for j in range(4):
                nc.tensor.transpose(pwu[:, j * P:(j + 1) * P], ident[:],
                                    ident[:])

        # ---------------- loads: Wq, Wk first (PE needs them earliest) -----
        wq_t = w32.tile([P, NDC, D], f32r, tag="w32", name="wq_t")   # slot0
        for dc in range(NDC):
            nc.sync.dma_start(wq_t[:, dc, :], wq_ap[dc * P:(dc + 1) * P, :])
        wk_t = w32.tile([P, NDC, D], f32r, tag="w32", name="wk_t")   # slot1
        for dc in range(NDC):
            nc.sync.dma_start(wk_t[:, dc, :], wk_ap[dc * P:(dc + 1) * P, :])

        # ---------------- W transposes: wXT[e%128, e//128, i] --------------
        def transpose_w(wsrc, wdst):
            # half-outer: all ec of dc 0-3 first, so PE never head-of-line
            # blocks on the second half of the W DMA
            for half in range(2):
                for ec in range(NDC):
                    pst = ps512.tile([P, QG], f32r, tag="mm", name="pstw")
                    for j in range(4):
                        dc = half * 4 + j
                        nc.tensor.transpose(
                            pst[:, j * P:(j + 1) * P],
                            wsrc[:, dc, ec * P:(ec + 1) * P], ident[:])
                    copy_engine(ec + half)(
                        wdst[:, ec, half * QG:(half + 1) * QG], pst[:])

        wqT = w32.tile([P, NDC, D], f32r, tag="w32", name="wqT")     # slot2
        transpose_w(wq_t, wqT)
        wkT = w32.tile([P, NDC, D], f32r, tag="w32", name="wkT")     # slot0
        transpose_w(wk_t, wkT)

        # ------- M' chains interleaved with x load + transpose -------------
        # M'[i, j] = sum_e Wk[i, e] Wq[j, e]; xT[d%128, d//128, s].
        # Interleaving keeps the x-chunk DMA pipeline draining (bufs=3
        # rotation frees a chunk right after its transposes) while PE chews
        # on M' accumulation chains.
        mp = w32.tile([P, NDC, D], f32r, tag="w32", name="mp")       # slot1
        xT = xtp.tile([P, NDC, S], f32r, tag="xt")

        def emit_x_chunk(sc):
            c = xck.tile([P, D], f32r, tag="xc", name="xc")
            nc.sync.dma_start(c[:], x_ap[sc * P:(sc + 1) * P, :])
            for half in range(2):
                pst = ps512.tile([P, QG], f32r, tag="mm", name="pstx")
                for j in range(4):
                    dc = half * 4 + j
                    nc.tensor.transpose(pst[:, j * P:(j + 1) * P],
                                        c[:, dc * P:(dc + 1) * P], ident[:])
                for j in range(4):
                    dc = half * 4 + j
                    copy_engine(sc + j)(xT[:, dc, sc * P:(sc + 1) * P],
                                        pst[:, j * P:(j + 1) * P])

        for t in range(16):
            ic, jg = t // 2, t % 2
            psm = ps512.tile([P, QG], f32, tag="mm", name="psm")
            for ec in range(NDC):
                nc.tensor.matmul(
                    psm[:], wkT[:, ec, ic * P:(ic + 1) * P],
                    wqT[:, ec, jg * QG:(jg + 1) * QG],
                    start=(ec == 0), stop=(ec == NDC - 1),
                )
            copy_engine(ic + jg)(mp[:, ic, jg * QG:(jg + 1) * QG], psm[:])
            emit_x_chunk(t)

        # Wv load (SP queue: after x chunks; lands before V(0) is needed)
        wv_t = w32.tile([P, NDC, D], f32r, tag="w32", name="wv_t")   # slot2
        for dc in range(NDC):
            nc.sync.dma_start(wv_t[:, dc, :], wv_ap[dc * P:(dc + 1) * P, :])
        vpp_t = w32.tile([P, NSC, D], bf16, tag="w32", name="vpp_t")  # slot0

        # ---------------- main fused loop over k-chunks ----------------
        at_t = None
        ec_t = None

        def emit_at_group(g):
            t = atp.tile([P, NDC, QG], f32r, tag="at", name=f"at{g}")
            for jc in range(NDC):
                ps = ps512.tile([P, QG], f32, tag="mm", name="psat")
                for ic in range(NDC):
                    nc.tensor.matmul(
                        ps[:], mp[:, ic, jc * P:(jc + 1) * P],
                        xT[:, ic, g * QG:(g + 1) * QG],
                        start=(ic == 0), stop=(ic == NDC - 1),
                    )
                copy_engine(jc)(t[:, jc, :], ps[:])
            return t

        def emit_c_group(j, ec_t, qis=(0, 1)):
            # out[q-chunk qc] = sum_{kc<=qc} E[kc block]^T @ V''[kc]
            for qi in qis:
                qc = 2 * j + qi
                pso = [ps512.tile([P, QG], f32, tag="mm", name=f"psc{eh}")
                       for eh in range(2)]
                for kc in range(qc + 1):
                    for eh in range(2):
                        nc.tensor.matmul(
                            pso[eh][:], ec_t[:, kc, qi * P:(qi + 1) * P],
                            vpp_t[:, kc, eh * QG:(eh + 1) * QG],
                            start=(kc == 0), stop=(kc == qc),
                        )
                st = osp.tile([P, D], f32, tag="os", name="ost")
                copy_engine(qi)(st[:, 0:QG], pso[0][:])
                copy_engine(qi + 1)(st[:, QG:D], pso[1][:])
                nc.scalar.dma_start(out_ap[qc * P:(qc + 1) * P, :], st[:])

        # C-group schedule: one group per kc, staggered to avoid the A'T
        # PSUM bursts at kc % 4 == 0; C(7) split so only qc=15 trails.
        c_sched = {2: 0, 5: 1, 6: 2, 9: 3, 10: 4, 13: 5, 14: 6}
        at_t = emit_at_group(0)
        for kc in range(NSC):
            g0 = kc // 4
            off0 = (kc % 4) * P
            if kc % 4 == 0 and kc > 0:
                at_t = emit_at_group(g0)
            # scores St[k, q] for q >= kc*128, q-512 groups; diagonal group
            # trimmed to >=256 columns (fp32r full-rate threshold)
            off_mm0 = min(off0, QG - 2 * P)
            pss = {}
            for qg in range(g0, NQG):
                off = off_mm0 if qg == g0 else 0
                ps = ps512.tile([P, QG], f32, tag="mm", name=f"pssc{qg}")
                pss[qg] = ps
                for jc in range(NDC):
                    nc.tensor.matmul(
                        ps[:, off:QG],
                        at_t[:, jc, off0:off0 + P],
                        xT[:, jc, qg * QG + off:(qg + 1) * QG],
                        start=(jc == 0), stop=(jc == NDC - 1),
                    )
            # C-group compute: only needs E rows <= kc-1 and V'' <= kc-1
            if kc in c_sched:
                emit_c_group(c_sched[kc], ec_t)
            if kc == NSC - 1:
                # prefetch E rows 0-14 for q-chunks 14/15 (row 15 after exp)
                ec_t = ecp.tile([P, NSC, CG], bf16, tag="ec", name="ec_t")
                nc.sync.dma_start(ec_t[:, 0:NSC - 1, :],
                                  e_dram[:, 0:NSC - 1, 7 * CG:8 * CG])
            # diagonal mask + global row max (negated max, min-combined)
            nc.vector.tensor_tensor(
                pss[g0][:, off0:off0 + P], pss[g0][:, off0:off0 + P], dmask[:],
                ALU.add,
            )
            nmall = tiny.tile([P, NQG], f32, tag="nmall")
            for qg in range(g0, NQG):
                off = off0 if qg == g0 else 0
                nc.vector.tensor_reduce(nmall[:, qg:qg + 1], pss[qg][:, off:QG],
                                        axis=AX.X, op=ALU.max, negate=True)
            negM = tiny.tile([P, 1], f32, tag="negM")
            nc.vector.tensor_reduce(negM[:], nmall[:, g0:NQG], axis=AX.X,
                                    op=ALU.min)
            # E = exp(s - M) -> bf16, row sums accumulated; one row DMA
            et = etp.tile([P, S], bf16, tag="et", name="et")
            sums = tiny.tile([P, NQG], f32, tag="sums")
            for qg in range(g0, NQG):
                off = off0 if qg == g0 else 0
                nc.scalar.activation(et[:, qg * QG + off:(qg + 1) * QG],
                                     pss[qg][:, off:QG], AF.Exp,
                                     bias=negM[:], scale=1.0,
                                     accum_out=sums[:, qg:qg + 1])
            nc.scalar.dma_start(e_dram[:, kc, kc * P:S], et[:, kc * P:S])
            # C-group E prefetch right behind this kc's E-row write
            if kc % 2 == 1 and kc < NSC - 1:
                j = (kc - 1) // 2
                ec_t = ecp.tile([P, NSC, CG], bf16, tag="ec", name="ec_t")
                nc.sync.dma_start(ec_t[:, 0:2 * j + 2, :],
                                  e_dram[:, 0:2 * j + 2, j * CG:(j + 1) * CG])
            ssum = tiny.tile([P, 1], f32, tag="ssum")
            nc.vector.tensor_reduce(ssum[:], sums[:, g0:NQG], axis=AX.X,
                                    op=ALU.add)
            nc.vector.reciprocal(rall[:, kc:kc + 1], ssum[:])
            # V[kc] = xT_kc^T @ Wv; V''[kc] = r * V -> bf16
            for eh in range(2):
                psv = ps512.tile([P, QG], f32, tag="mm", name="psv")
                for dc in range(NDC):
                    nc.tensor.matmul(
                        psv[:], xT[:, dc, kc * P:(kc + 1) * P],
                        wv_t[:, dc, eh * QG:(eh + 1) * QG],
                        start=(dc == 0), stop=(dc == NDC - 1),
                    )
                nc.vector.tensor_scalar_mul(
                    vpp_t[:, kc, eh * QG:(eh + 1) * QG], psv[:],
                    rall[:, kc:kc + 1],
                )
            if kc == NSC - 1:
                # qc=14 needs only E rows 0-14 (prefetched above) and r<=14
                emit_c_group(7, ec_t, qis=(0,))
                nc.sync.dma_start(ec_t[:, NSC - 1:NSC, :],
                                  e_dram[:, NSC - 1:NSC, 7 * CG:8 * CG])

        emit_c_group(7, ec_t, qis=(1,))


_PROGRAMS = {}


def _get_program(n_repeats=1):
    if n_repeats not in _PROGRAMS:
        from concourse import bacc
        import concourse.tile as tile
        import concourse.mybir as mybir

        f32 = mybir.dt.float32
        nc = bacc.Bacc("TRN2", target_bir_lowering=False, debug=False,
                       enable_asserts=False, num_devices=NCORES)
        x_ap = nc.dram_tensor("x_local", (S, D), mybir.dt.float32r, kind="ExternalInput").ap()
        wq_ap = nc.dram_tensor("wq", (D, D), mybir.dt.float32r, kind="ExternalInput").ap()
        wk_ap = nc.dram_tensor("wk", (D, D), mybir.dt.float32r, kind="ExternalInput").ap()
        wv_ap = nc.dram_tensor("wv", (D, D), mybir.dt.float32r, kind="ExternalInput").ap()
        out_ap = nc.dram_tensor("out_local", (S, D), f32, kind="ExternalOutput").ap()
        with tile.TileContext(nc) as tc:
            if n_repeats == 1:
                build_body(tc, out_ap, x_ap, wq_ap, wk_ap, wv_ap)
            else:
                with tc.For_i(0, n_repeats, 1):
                    build_body(tc, out_ap, x_ap, wq_ap, wk_ap, wv_ap)
        nc.compile()
        _PROGRAMS[n_repeats] = nc
    return _PROGRAMS[n_repeats]


def run(x, Wq, Wk, Wv, trace=False, **spmd_kwargs):
    from concourse import bass_utils

    nc = _get_program()
    x = np.ascontiguousarray(np.asarray(x, dtype=np.float32))
    Wq = np.ascontiguousarray(np.asarray(Wq, dtype=np.float32))
    Wk = np.ascontiguousarray(np.asarray(Wk, dtype=np.float32))
    Wv = np.ascontiguousarray(np.asarray(Wv, dtype=np.float32))
    in_maps = [
        {"x_local": np.ascontiguousarray(x[i]), "wq": Wq, "wk": Wk, "wv": Wv}
        for i in range(NCORES)
    ]
    res = bass_utils.run_bass_kernel_spmd(
        nc, in_maps, core_ids=list(range(NCORES)), trace=trace, **spmd_kwargs
    )
    out = np.stack([r["out_local"] for r in res.results]).astype(np.float32)
    return out, res


def kernel(x, Wq, Wk, Wv):
    out, _ = run(x, Wq, Wk, Wv, trace=False)
    return out
